# revision 78
# baseline (speedup 1.0000x reference)
"""BiasAttention TRN2 kernel — q-sharded, fp8 z, baseline loop structure.

Known-good probe variant (the 114us run): fp8-e3m4 z, gk=64 DMA groups,
HAM warmup, original S-prologue + in-loop bias/add/exp/transpose/AV.
"""

import sys

if "/opt/trn_rl_repo" not in sys.path:
    sys.path.insert(0, "/opt/trn_rl_repo")

import ml_dtypes
import numpy as np

import concourse.bass as bass
import concourse.mybir as mybir
from concourse import bacc
from concourse.bass_utils import run_bass_kernel_spmd
from concourse.masks import make_identity
from concourse.tile import TileContext

P = 128
H = 8
D = 32
CQ = 256
CKV = 256
BD = 128
NQ = 1024
NCORES = 8
NQC = NQ // NCORES
SCALE = D ** (-0.5)

GK = 64
FP = mybir.dt.float32
BF = mybir.dt.bfloat16
F8 = mybir.dt.float8e3
NP_BF = ml_dtypes.bfloat16
NP_F8 = ml_dtypes.float8_e3m4

Z_SCALE = 2.0
WB_SCALE = 32.0
BIAS_SCALE = Z_SCALE * WB_SCALE


def build_program(nk=1024, gk=GK):
    kc_n = nk // P
    ng = nk // gk
    add = mybir.AluOpType.add
    mult = mybir.AluOpType.mult

    nc = bacc.Bacc("TRN2", target_bir_lowering=False, debug=False,
                   num_devices=NCORES)

    # All weights/activations arrive pre-laid-out host-side in the exact
    # [partition, ...] shape SBUF wants — trivial contiguous DMA
    # descriptors (rearranged APs cost 0.6-3.3us EACH to generate on the
    # serial sync queue and were delaying the whole pipeline by ~40us).
    # bb is omitted on purpose: it is constant along the softmax axis, so
    # softmax(S + bias + bb) == softmax(S + bias).
    zT = nc.dram_tensor("zT", [ng, BD, gk, NQC], F8, kind="ExternalInput")
    # bf16 weights/activations packed into ONE blob (cols: wq 512,
    # wkv 1024, xq 256, xkv 2048), fp32 into another (bq 2, bkvK 2,
    # wp 512), plus one single-partition row (bkvV 256 | bp 256) — one
    # cheap contiguous descriptor-gen each instead of nine.
    Wbf = nc.dram_tensor("Wbf", [P, 3840], BF, kind="ExternalInput")
    Wfp = nc.dram_tensor("Wfp", [P, 516], FP, kind="ExternalInput")
    Vrow = nc.dram_tensor("Vrow", [1, 2 * CQ], FP, kind="ExternalInput")
    Wb = nc.dram_tensor("Wb", [BD, H], F8, kind="ExternalInput")
    y = nc.dram_tensor("y", [NQC, CQ], FP, kind="ExternalOutput")

    with TileContext(nc) as tc:
        with (
            tc.tile_pool(name="const", bufs=1) as const,
            tc.tile_pool(name="zpool", bufs=12) as zpool,
            tc.tile_pool(name="xpool", bufs=3) as xpool,
            tc.tile_pool(name="epool", bufs=3) as epool,
            tc.tile_pool(name="atpool", bufs=4) as atpool,
            tc.tile_pool(name="proj_ps", bufs=2, space="PSUM") as proj_ps,
            tc.tile_pool(name="b_ps", bufs=3, space="PSUM") as b_psp,
            tc.tile_pool(name="t_ps", bufs=2, space="PSUM") as t_psp,
            tc.tile_pool(name="o_ps", bufs=1, space="PSUM") as o_psp,
        ):
            wb_sb = const.tile([P, H], F8)
            nc.sync.dma_start(wb_sb, Wb[:])
            # First 3 z groups lead the ring for a head start; weights are
            # queued before the rest of the z stream so a buffer-slot wait
            # on the sync queue can never starve them (12 upfront = pool
            # depth, so none of these waits; the last 4 issue in-loop).
            zlist = []

            def z_fetch(gidx):
                z_sb = zpool.tile([P, GK, NQC], F8, tag="z", name=f"zg{gidx}")
                nc.sync.dma_start(z_sb, zT[gidx])
                zlist.append(z_sb)

            for gidx in range(4):
                z_fetch(gidx)

            # weight blobs ride the GPSIMD (SWDGE) DMA queue so they move
            # concurrently with the z stream on the sync queue — z can
            # start at ~10us AND the weights land by ~12us.
            wbf_sb = const.tile([P, 3840], BF)
            nc.gpsimd.dma_start(out=wbf_sb, in_=Wbf[:])
            wfp_sb = const.tile([P, 516], FP)
            nc.gpsimd.dma_start(out=wfp_sb, in_=Wfp[:])
            vrow_sb = const.tile([1, 2 * CQ], FP)
            nc.gpsimd.dma_start(out=vrow_sb, in_=Vrow[:])
            wq_sb = wbf_sb[:, 0:512].rearrange("p (o m) -> p o m", o=2)
            wkv_sb = wbf_sb[:, 512:1536].rearrange("p (o m) -> p o m", o=2)
            xqT_sb = wbf_sb[:, 1536:1792].rearrange("p (o m) -> p o m", o=2)
            xkvT_sb = wbf_sb[:, 1792:3840].rearrange("p (o m) -> p o m", o=2)
            bq_sb = wfp_sb[:, 0:2]
            bkvK_sb = wfp_sb[:, 2:4]
            wp_sb = wfp_sb[:, 4:516].rearrange("p (o m) -> p o m", o=2)
            bkvV_sb = vrow_sb[:, 0:H * D]
            bp_sb = vrow_sb[:, CQ:2 * CQ]
            ident = const.tile([P, P], FP)
            make_identity(nc, ident)
            ident_bf = const.tile([P, P], BF)
            make_identity(nc, ident_bf)
            ones_row = const.tile([1, P], FP)
            nc.vector.memset(ones_row, 1.0)
            # rest of the upfront z prefetch, behind the weight DMAs
            for gidx in range(4, 12):
                z_fetch(gidx)

            # HAM warmup: dense dummy matmuls while the first DMAs are in
            # flight; depends only on a vector-engine memset (gpsimd
            # identity takes ~7us to start).
            warm_sb = const.tile([P, P], BF)
            nc.vector.memset(warm_sb, 0.5)
            warm_ps = proj_ps.tile([P, 512], FP, tag="proj", name="warm")
            for w in range(48):
                nc.tensor.matmul(warm_ps[:, :P], lhsT=warm_sb, rhs=warm_sb,
                                 start=(w == 0), stop=(w == 47))

            vaug_sb = const.tile([P, kc_n, H, D + 1], BF)
            nc.vector.memset(vaug_sb, 1.0)

            qT_sb = const.tile([P, 2, NQC], BF)
            with tc.tile_wait_until(1.02):
                for m in range(2):
                    ps = proj_ps.tile([P, 512], FP, tag="proj")
                    for c in range(2):
                        nc.tensor.matmul(ps[:, :NQC],
                                         lhsT=wq_sb[:, c, m * P:(m + 1) * P],
                                         rhs=xqT_sb[:, c, :],
                                         start=(c == 0), stop=(c == 1))
                    nc.vector.tensor_scalar(qT_sb[:, m, :], ps[:, :NQC],
                                            bq_sb[:, m:m + 1],
                                            SCALE * BIAS_SCALE, add, mult)

            kT_sb = const.tile([P, 2, nk], BF)
            with tc.tile_wait_until(1.05):
                for m in range(2):
                    for nh in range((nk + 511) // 512):
                        nn_ = min(512, nk - nh * 512)
                        ps = proj_ps.tile([P, 512], FP, tag="proj")
                        for c in range(2):
                            nc.tensor.matmul(
                                ps[:, :nn_],
                                lhsT=wkv_sb[:, c, m * P:(m + 1) * P],
                                rhs=xkvT_sb[:, c, nh * 512:nh * 512 + nn_],
                                start=(c == 0), stop=(c == 1))
                        # alternate the +bkvK copy between DVE and ACT
                        if (m * 2 + nh) % 2 == 0:
                            nc.vector.tensor_scalar(
                                kT_sb[:, m, nh * 512:nh * 512 + nn_],
                                ps[:, :nn_], bkvK_sb[:, m:m + 1], None, add)
                        else:
                            nc.scalar.activation(
                                kT_sb[:, m, nh * 512:nh * 512 + nn_],
                                ps[:, :nn_],
                                mybir.ActivationFunctionType.Identity,
                                bias=bkvK_sb[:, m:m + 1])

            s_sb = const.tile([P, H, nk], BF)

            def emit_s(h, nh):
                """One 512-key S slab for head h: QK matmul + bias copy."""
                r0 = (h % 4) * 32
                ps = proj_ps.tile([P, 512], FP, tag="proj", name="qk_ps")
                nc.tensor.matmul(ps[:, :512],
                                 lhsT=qT_sb[r0:r0 + 32, h // 4, :],
                                 rhs=kT_sb[r0:r0 + 32, h // 4,
                                           nh * 512:nh * 512 + 512],
                                 start=True, stop=True,
                                 tile_position=(r0, 0))
                if (h * 2 + nh) % 2 == 0:
                    nc.scalar.activation(
                        s_sb[:, h, nh * 512:nh * 512 + 512], ps[:, :512],
                        mybir.ActivationFunctionType.Copy)
                else:
                    nc.vector.tensor_copy(
                        s_sb[:, h, nh * 512:nh * 512 + 512], ps[:, :512])

            # keys 0-511 behind chunk 0's z matmuls (the adds of chunk 0
            # are the late consumers and b_ps has 3 banks of runway);
            # keys 512-1023 sprinkle into later iterations (needed from
            # chunk 4 on).
            with tc.tile_wait_until(1.08):
                for h in range(H):
                    emit_s(h, 0)

            # V(kc) is only needed by AV(kc) at virtual ~(2.75+kc); spread
            # the projections across the early chunks to fill PE slack.
            for kc in range(kc_n):
                with tc.tile_wait_until(1.2 + 0.45 * kc):
                    ps = proj_ps.tile([P, 512], FP, tag="proj", name="v_ps")
                    for c in range(2):
                        nc.tensor.matmul(
                            ps[:, :H * D],
                            lhsT=xkvT_sb[:, c, kc * P:(kc + 1) * P],
                            rhs=wkv_sb[:, c, H * D:2 * H * D],
                            start=(c == 0), stop=False)
                    nc.tensor.matmul(ps[:, :H * D], lhsT=ones_row,
                                     rhs=bkvV_sb, start=False, stop=True)
                    nc.scalar.activation(
                        vaug_sb[:, kc, :, 0:D],
                        ps[:, :H * D].rearrange("p (h d) -> p h d", h=H),
                        mybir.ActivationFunctionType.Copy)

            o_ps = o_psp.tile([P, H * (D + 1)], FP)
            HKT = 64

            def emit_t(kc, x_sb):
                """Transposes + at-copies for chunk kc (x_sb is ready)."""
                ats = []
                for hg in range(2):
                    t_ps = t_psp.tile([P, 4, P], BF, tag="t")
                    for hl in range(4):
                        nc.tensor.transpose(t_ps[:, hl, :],
                                            x_sb[:, hg * 4 + hl, :], ident_bf)
                    at_sb = atpool.tile([P, 4, P], BF, tag="at")
                    nc.vector.tensor_copy(at_sb, t_ps)
                    ats.append(at_sb)
                return ats

            def emit_av(kc, ats):
                for hg in range(2):
                    for hl in range(4):
                        h = hg * 4 + hl
                        nc.tensor.matmul(
                            o_ps[:, h * (D + 1):(h + 1) * (D + 1)],
                            lhsT=ats[hg][:, hl, :], rhs=vaug_sb[:, kc, h, :],
                            start=(kc == 0 and h == 0),
                            stop=(kc == kc_n - 1 and h == H - 1))

            def emit_half(kc, hf, x_sb):
                b_ps = b_psp.tile([P, HKT * H], FP, tag="b")
                z_sb = zlist[kc * 2 + hf]
                for t in range(HKT):
                    nc.tensor.matmul(b_ps[:, t * H:(t + 1) * H],
                                     lhsT=z_sb[:, t, :], rhs=wb_sb,
                                     start=(t == 0), stop=(t == HKT - 1))
                e_sb = epool.tile([P, H, HKT], FP, tag="e")
                nc.vector.tensor_tensor(
                    e_sb,
                    s_sb[:, :, kc * P + hf * HKT:kc * P + (hf + 1) * HKT],
                    b_ps.rearrange("p (kt h) -> p h kt", h=H), add)
                nc.scalar.activation(x_sb[:, :, hf * HKT:(hf + 1) * HKT],
                                     e_sb,
                                     mybir.ActivationFunctionType.Exp,
                                     scale=1.0 / BIAS_SCALE)

            # Steady-state PE order per chunk: zA(kc), T(kc-1), zB(kc),
            # AV(kc-1) — each half's add+exp completes under the opposite
            # half's z matmuls, so the PE never waits on the exp chain.
            # The Tile scheduler's cost model mispredicts (no LDWEIGHTS
            # model, serial DMA), so the order is pinned with manual
            # virtual-time stamps (tile_wait_until in fake "ms" units).
            prev = None
            for kc in range(kc_n):
                base = 1.0 + kc
                x_sb = xpool.tile([P, H, P], BF, tag="x")
                with tc.tile_wait_until(base):
                    for gidx in (kc * 2 + 12, kc * 2 + 13):
                        if gidx < ng:
                            z_fetch(gidx)
                    emit_half(kc, 0, x_sb)
                ats = None
                if prev is not None:
                    with tc.tile_wait_until(base + 0.25):
                        ats = emit_t(kc - 1, prev)
                # late S slabs (keys 512-1023) fill early-loop PE slack
                if kc in (1, 2):
                    with tc.tile_wait_until(base + 0.8):
                        for h in range(4 * (kc - 1), 4 * (kc - 1) + 4):
                            emit_s(h, 1)
                with tc.tile_wait_until(base + (0.15 if kc == 0 else 0.5)):
                    emit_half(kc, 1, x_sb)
                if ats is not None:
                    with tc.tile_wait_until(base + 0.75):
                        emit_av(kc - 1, ats)
                prev = x_sb
            with tc.tile_wait_until(1.0 + kc_n):
                ats = emit_t(kc_n - 1, prev)
                emit_av(kc_n - 1, ats)

            with tc.tile_wait_until(2.0 + kc_n):
                recip_sb = const.tile([P, H], FP)
                nc.vector.reciprocal(
                    recip_sb,
                    o_ps.rearrange("p (h d) -> p h d", h=H)[:, :, D])
                o_sb = const.tile([P, 2, P], FP)
                for h in range(H):
                    nc.vector.tensor_scalar(
                        o_sb[:, h // 4, (h % 4) * 32:(h % 4) * 32 + 32],
                        o_ps[:, h * (D + 1):h * (D + 1) + D],
                        recip_sb[:, h:h + 1], None, mult)
                oT_sb = const.tile([P, 2, P], FP)
                for m in range(2):
                    t_full = proj_ps.tile([P, 512], FP, tag="proj",
                                          name="t_full")
                    t_ps = t_full[:, :P]
                    nc.tensor.transpose(t_ps, o_sb[:, m, :], ident)
                    nc.vector.tensor_copy(oT_sb[:, m, :], t_ps)
                ps = proj_ps.tile([P, 512], FP, tag="proj")
                for m in range(2):
                    nc.tensor.matmul(ps[:, :CQ], lhsT=oT_sb[:, m, :],
                                     rhs=wp_sb[:, m, :], start=(m == 0),
                                     stop=False)
                nc.tensor.matmul(ps[:, :CQ], lhsT=ones_row, rhs=bp_sb,
                                 start=False, stop=True)
                y_sb = const.tile([P, CQ], FP)
                nc.vector.tensor_copy(y_sb, ps[:, :CQ])
                nc.sync.dma_start(y[:], y_sb)

    nc.compile()
    return nc


def _p2(a, dt):
    """[(o p), m...] -> [p, o, m...] contiguous, cast to dt."""
    a = np.asarray(a)
    return np.ascontiguousarray(
        a.reshape(2, P, *a.shape[1:]).transpose(1, 0, *range(2, a.ndim + 1))
    ).astype(dt)


def prep_inputs(x_q, x_kv, z, Wq, bq, Wkv, bkv, Wb, bb, Wp, bp,
                nk=1024, gk=GK):
    ng = nk // gk
    wbf = np.empty((P, 3840), dtype=NP_BF)
    wbf[:, 0:512] = _p2(Wq, NP_BF).reshape(P, 512)
    wbf[:, 512:1536] = _p2(Wkv, NP_BF).reshape(P, 1024)
    wbf[:, 1792:3840] = _p2(
        np.ascontiguousarray(x_kv[0].T), NP_BF).reshape(P, 2048)
    wfp = np.empty((P, 516), dtype=np.float32)
    wfp[:, 0:2] = _p2(np.asarray(bq, dtype=np.float32), np.float32)
    wfp[:, 2:4] = _p2(np.asarray(bkv[:H * D], dtype=np.float32), np.float32)
    wfp[:, 4:516] = _p2(Wp, np.float32).reshape(P, 512)
    vrow = np.concatenate([np.asarray(bkv[H * D:], dtype=np.float32),
                           np.asarray(bp, dtype=np.float32)]).reshape(1, -1)
    shared = dict(
        Wfp=wfp,
        Vrow=vrow,
        Wb=(np.asarray(Wb, dtype=np.float32) * WB_SCALE).astype(NP_F8))
    in_maps = []
    for i in range(NCORES):
        qs = i * NQC
        zi = z[0, qs:qs + NQC]
        zi = zi.reshape(NQC, ng, gk, BD).transpose(1, 3, 2, 0)
        wbf_i = wbf.copy()
        wbf_i[:, 1536:1792] = _p2(
            np.ascontiguousarray(x_q[0, qs:qs + NQC].T), NP_BF
        ).reshape(P, 256)
        in_maps.append(dict(
            zT=(np.ascontiguousarray(zi) * np.float32(Z_SCALE)
                ).astype(NP_F8),
            Wbf=wbf_i,
            **shared,
        ))
    return in_maps


_NC_CACHE = {}


def kernel(x_q, x_kv, z, Wq, bq, Wkv, bkv, Wb, bb, Wp, bp):
    key = "full"
    if key not in _NC_CACHE:
        _NC_CACHE[key] = build_program()
    nc = _NC_CACHE[key]
    in_maps = prep_inputs(x_q, x_kv, z, Wq, bq, Wkv, bkv, Wb, bb, Wp, bp)
    res = run_bass_kernel_spmd(nc, in_maps, list(range(NCORES)))
    out = np.empty((1, NQ, CQ), dtype=np.float32)
    for i in range(NCORES):
        out[0, i * NQC:(i + 1) * NQC, :] = res.results[i]["y"]
    return out
